# revision 1
# baseline (speedup 1.0000x reference)
"""EfficientAttention Trainium2 Bass kernel.

Reference computation (per token t, H=16 heads, hd=64):
  Q = x @ Wq.T ; K = x @ Wk.T ; V = x @ Wv.T        (d = 1024)
  sK = softmax over heads of K^T      : sK[d,h] = expK[h,d] / rk[d]
  tran_V = sK @ V                      (64 x 64)
  out = softmax(Q, axis=-1) @ tran_V   (16 x 64) -> flatten 1024

Equivalent form used here (per token):
  A^T[h',h] = sum_d expQ[h,d] * sK[d,h']    (then scale rows by 1/rq[h])
  out[h,:]  = sum_h' A[h,h'] * V[h',:]

Sharding: data-parallel over the 16384 tokens across 8 cores (2048 each).
Weights replicated, pre-transposed + bf16-cast on host.

Device layout: tokens on SBUF partitions (128-token tiles).
  PE   : x-tile transposes + the three 1024x1024 projections (bf16)
  ACT  : psum->sbuf evictions fused with exp for Q,K
  DVE  : softmax normalizers + the per-token small matmuls
"""

import numpy as np
import ml_dtypes
from contextlib import ExitStack

import concourse.bass as bass
import concourse.mybir as mybir
import concourse.tile as tile
from concourse import bacc
from concourse.bass_utils import run_bass_kernel_spmd

DIMS = 1024
HEADS = 16
HD = 64
N_CORES = 8
B, L = 4, 4096
TOKENS = B * L
TOK_PER_CORE = TOKENS // N_CORES  # 2048
P = 128                           # tokens per tile (SBUF partitions)
N_TILES = TOK_PER_CORE // P       # 16

FP32 = mybir.dt.float32
BF16 = mybir.dt.bfloat16

_COMPILED = {}


def _register_mac_scan():
    """Custom fused DVE op: out = prefix-sum of (in0 * in1) along the stream.

    Used for the per-token second matmul: per head h, stream (e outer,
    h' inner) of V[t,(h',e)] * A[t,(h,h')]; segment sums are recovered by
    differencing the prefix at 16-element boundaries.
    """
    from concourse.dve_spec import Spec, Src0, Src1, scan, AluOp, lower
    from concourse import dve_ops
    from concourse.dve_uop import DveOpSpec
    from concourse.bass import dve_ver_for

    if "MAC_SCAN" in dve_ops._SUB_OPCODE_FOR_NAME:
        return next(o for o in dve_ops.OPS if o.name == "MAC_SCAN")

    spec = Spec(body=scan(AluOp.ADD, Src0 * Src1),
                reference=lambda in0, in1: np.cumsum(
                    in0.astype(np.float32) * in1.astype(np.float32), axis=-1))
    row = 1 + len(dve_ops.OPS)
    shas = {}
    for ver in ("v3", "v4"):
        tmp = DveOpSpec(name="MAC_SCAN", opcode=row,
                        uops=lower(spec, ver=ver), rd1_en=True)
        shas[ver] = tmp.sha(ver)
    op = dve_ops.DveOp("MAC_SCAN", spec, subdim=False, uops_sha=shas)
    dve_ops.OPS.append(op)
    dve_ops.CUSTOM_DVE_SPECS["MAC_SCAN"] = spec
    dve_ops._SUB_OPCODE_FOR_NAME["MAC_SCAN"] = row
    return op


MAC_SCAN = _register_mac_scan()


def _build_kernel():
    nc = bacc.Bacc("TRN2", target_bir_lowering=False)

    x_in = nc.dram_tensor("x", [TOK_PER_CORE, DIMS], BF16, kind="ExternalInput")
    wq_in = nc.dram_tensor("wq", [DIMS, DIMS], BF16, kind="ExternalInput")
    wk_in = nc.dram_tensor("wk", [DIMS, DIMS], BF16, kind="ExternalInput")
    wv_in = nc.dram_tensor("wv", [DIMS, DIMS], BF16, kind="ExternalInput")
    ident_in = nc.dram_tensor("ident", [P, P], BF16, kind="ExternalInput")
    out_d = nc.dram_tensor("out", [TOK_PER_CORE, DIMS], FP32, kind="ExternalOutput")

    with tile.TileContext(nc) as tc, ExitStack() as ctx:
        consts = ctx.enter_context(tc.tile_pool(name="consts", bufs=1))
        wpool = ctx.enter_context(tc.tile_pool(name="weights", bufs=1))
        xpool = ctx.enter_context(tc.tile_pool(name="x", bufs=N_TILES))
        xtpool = ctx.enter_context(tc.tile_pool(name="xt", bufs=2))
        smpool = ctx.enter_context(tc.tile_pool(name="sm", bufs=3))
        slabpool = ctx.enter_context(tc.tile_pool(name="slab", bufs=2))
        mmpool = ctx.enter_context(tc.tile_pool(name="mm", bufs=4))
        opool = ctx.enter_context(tc.tile_pool(name="outs", bufs=3))
        # PSUM budget (8 banks): pp(proj)=2, tp(xT+extract slabs)=2x2,
        # pa(A + A-back)=2
        ps_pp = ctx.enter_context(tc.tile_pool(name="ps_pp", bufs=2, space="PSUM"))
        ps_tp = ctx.enter_context(tc.tile_pool(name="ps_tp", bufs=4, space="PSUM"))
        ps_pa = ctx.enter_context(tc.tile_pool(name="ps_pa", bufs=2, space="PSUM"))

        ident = consts.tile([P, P], BF16)
        nc.sync.dma_start(ident[:], ident_in[:])

        xts = {}
        for pre in range(2):
            xt0 = xpool.tile([P, DIMS], BF16, tag="xt", name=f"xpre{pre}")
            nc.sync.dma_start(xt0[:], x_in[pre * P:(pre + 1) * P, :])
            xts[pre] = xt0

        ws = {}
        for name, w_in in (("q", wq_in), ("k", wk_in), ("v", wv_in)):
            w = wpool.tile([P, 8 * DIMS], BF16, tag=f"w{name}")
            for c in range(8):
                nc.sync.dma_start(w[:, c * DIMS:(c + 1) * DIMS],
                                  w_in[c * P:(c + 1) * P, :])
            ws[name] = w

        for it in range(N_TILES):
            # 1) load x tile [128 tokens, 1024] bf16
            if it in xts:
                xt = xts[it]
            else:
                xt = xpool.tile([P, DIMS], BF16)
                nc.sync.dma_start(xt[:], x_in[it * P:(it + 1) * P, :])

            # 2) transpose x tile -> xT [j, t] stored as 8 chunks side by side
            xT = xtpool.tile([P, DIMS], BF16)
            for c in range(8):
                tp = ps_tp.tile([P, P], BF16, tag="tp", name=f"tp{it}_{c}")
                nc.tensor.transpose(tp[:], xt[:, c * P:(c + 1) * P], ident[:])
                nc.scalar.copy(xT[:, c * P:(c + 1) * P], tp[:])

            # 3) projections; per proj 2 psum banks, evicted via ACT
            #    (exp fused for Q,K; plain copy for V)
            expq = smpool.tile([P, DIMS], BF16, tag="expq")
            expk = smpool.tile([P, DIMS], BF16, tag="expk")
            vt = smpool.tile([P, DIMS], BF16, tag="vt")
            for pname, dst, func in (
                ("q", expq, mybir.ActivationFunctionType.Exp),
                ("k", expk, mybir.ActivationFunctionType.Exp),
                ("v", vt, None),
            ):
                w = ws[pname]
                for nb in range(2):
                    pp = ps_pp.tile([P, 512], FP32, tag="pp",
                                    name=f"pp{it}_{pname}{nb}")
                    for c in range(8):
                        nc.tensor.matmul(
                            pp[:],
                            lhsT=xT[:, c * P:(c + 1) * P],
                            rhs=w[:, c * DIMS + nb * 512: c * DIMS + nb * 512 + 512],
                            start=(c == 0), stop=(c == 7),
                        )
                    sl = slice(nb * 512, nb * 512 + 512)
                    if func is None:
                        nc.scalar.copy(dst[:, sl], pp[:])
                    else:
                        nc.scalar.activation(dst[:, sl], pp[:], func)

            # 4) softmax normalizers on DVE
            rq = smpool.tile([P, HEADS], FP32, tag="rq")       # sum_d expQ[h,d]
            nc.vector.reduce_sum(rq[:], expq[:].rearrange("p (h d) -> p h d", d=HD),
                                 axis=mybir.AxisListType.X)
            # rk[d] = sum_h expK[t,(h,d)] via contiguous halving adds
            t1 = smpool.tile([P, 512], BF16, tag="t1")
            nc.vector.tensor_add(t1[:], expk[:, 0:512], expk[:, 512:1024])
            t2 = smpool.tile([P, 256], BF16, tag="t2")
            nc.vector.tensor_add(t2[:], t1[:, 0:256], t1[:, 256:512])
            t3 = smpool.tile([P, 128], BF16, tag="t3")
            nc.vector.tensor_add(t3[:], t2[:, 0:128], t2[:, 128:256])
            rk = smpool.tile([P, HD], FP32, tag="rk")
            nc.vector.tensor_add(rk[:], t3[:, 0:HD], t3[:, HD:128])
            rqi = smpool.tile([P, HEADS], FP32, tag="rqi")
            nc.vector.reciprocal_approx_fast(rqi[:], rq[:])
            rki = smpool.tile([P, HD], FP32, tag="rki")
            nc.vector.reciprocal_approx_fast(rki[:], rk[:])
            rkib = smpool.tile([P, HD], BF16, tag="rkib")
            nc.scalar.copy(rkib[:], rki[:])

            # 5) sQ'[t,(h,d)] = expQ * rqi[h] * rki[d]  (both softmax scales
            #    folded into the Q side; K side stays raw expK)
            sqt = smpool.tile([P, DIMS], BF16, tag="sqt")
            rkib_b = rkib[:].unsqueeze(1).broadcast_to([P, HEADS, HD])
            nc.vector.tensor_mul(sqt[:].rearrange("p (h d) -> p h d", d=HD),
                                 expq[:].rearrange("p (h d) -> p h d", d=HD),
                                 rkib_b)

            # 6) extraction: per-head PE transposes -> feature-on-partition
            #    slabs QS/KS [64 d, 16 heads x 128 tokens] bf16
            slabs = {}
            for sname, srct in (("qs", sqt), ("ks", expk)):
                slab = slabpool.tile([HD, HEADS * P], BF16, tag=sname)
                for b in range(2):
                    ep = ps_tp.tile([HD, 8 * P], BF16, tag="tp",
                                    name=f"ep{it}_{sname}{b}")
                    for hh in range(8):
                        h = 8 * b + hh
                        nc.tensor.transpose(
                            ep[:, hh * P:(hh + 1) * P],
                            srct[:, h * HD:(h + 1) * HD],
                            ident[:])
                    nc.scalar.copy(slab[:, b * 8 * P:(b + 1) * 8 * P], ep[:])
                slabs[sname] = slab

            # 7) mm1 on PE: per token A[h',h] = sum_d KS[d,h'] * QS[d,h]
            #    out [16,16] per token, 32 tokens per psum bank, evicted bf16
            aev = mmpool.tile([HEADS, 2048], BF16, tag="aev")
            for bk in range(4):
                pa = ps_pa.tile([HEADS, 512], FP32, tag="pa",
                                name=f"pa{it}_{bk}")
                for ts in range(32):
                    t = 32 * bk + ts
                    lhsT = slabs["ks"][:, t::P]
                    rhs = slabs["qs"][:, t::P]
                    nc.tensor.matmul(
                        pa[:, ts * HEADS:(ts + 1) * HEADS],
                        lhsT=lhsT, rhs=rhs,
                        start=True, stop=True)
                nc.scalar.copy(aev[:, bk * 512:(bk + 1) * 512], pa[:])

            # 8) A back to token-partitions: per h transpose
            #    [16 h', 128 t] -> [128 t, 16 h'], pack into at [128, 256]
            at = mmpool.tile([P, HEADS * HEADS], BF16, tag="at")
            ab = ps_pa.tile([P, HEADS * HEADS], BF16, tag="pa",
                            name=f"ab{it}")
            for h in range(HEADS):
                src = aev[:, h::HEADS]
                nc.tensor.transpose(
                    ab[:, h * HEADS:(h + 1) * HEADS],
                    src, ident[0:HEADS, 0:HEADS])
            nc.scalar.copy(at[:], ab[:])
            rqi_b = rqi[:].unsqueeze(2).broadcast_to([P, HEADS, HEADS])
            nc.vector.tensor_mul(at[:].rearrange("p (h k) -> p h k", k=HEADS),
                                 at[:].rearrange("p (h k) -> p h k", k=HEADS),
                                 rqi_b)

            # 9) mm2 via MAC-scan: out[t,(h,e)] = sum_h' A[t,(h,h')]*V[t,(h',e)]
            ot = opool.tile([P, DIMS], FP32, tag="ot")
            sc = mmpool.tile([P, HEADS * HD + 1], FP32, tag="sc")
            nc.vector.memset(sc[:, 0:1], 0.0)
            for h in range(HEADS):
                v_view = vt[:].rearrange("p (hp e) -> p e hp", e=HD)
                a_view = at[:, h * HEADS:(h + 1) * HEADS] \
                    .unsqueeze(1).broadcast_to([P, HD, HEADS])
                nc.vector._custom_dve(
                    MAC_SCAN,
                    out=sc[:, 1:HEADS * HD + 1],
                    in0=v_view,
                    in1=a_view,
                )
                nc.vector.tensor_sub(
                    ot[:, h * HD:(h + 1) * HD],
                    sc[:, HEADS:HEADS * HD + 1:HEADS],
                    sc[:, 0:HEADS * HD:HEADS],
                )
            # 10) store
            nc.sync.dma_start(out_d[it * P:(it + 1) * P, :], ot[:])

    nc.compile()
    return nc


def kernel(input_seq_embs, W_Q, W_K, W_V):
    x = np.asarray(input_seq_embs, dtype=np.float32).reshape(TOKENS, DIMS)
    x_bf = x.astype(ml_dtypes.bfloat16)
    # torch Linear computes x @ W.T; our matmul wants rhs = W.T laid out
    # [contraction j, out i] == W_Q.T, which is exactly W.T in row-major.
    wq = np.ascontiguousarray(np.asarray(W_Q, np.float32).T).astype(ml_dtypes.bfloat16)
    wk = np.ascontiguousarray(np.asarray(W_K, np.float32).T).astype(ml_dtypes.bfloat16)
    wv = np.ascontiguousarray(np.asarray(W_V, np.float32).T).astype(ml_dtypes.bfloat16)
    ident = np.eye(P, dtype=ml_dtypes.bfloat16)

    if "nc" not in _COMPILED:
        _COMPILED["nc"] = _build_kernel()
    nc = _COMPILED["nc"]

    in_maps = []
    for c in range(N_CORES):
        shard = np.ascontiguousarray(x_bf[c * TOK_PER_CORE:(c + 1) * TOK_PER_CORE])
        in_maps.append({"x": shard, "wq": wq, "wk": wk, "wv": wv, "ident": ident})

    import os
    trace = bool(int(os.environ.get("KERNEL_PROFILE", "0")))
    kw = {}
    if trace:
        kw = dict(trace=True, tmpdir=os.environ.get("KERNEL_TRACE_DIR") or None)
    res = run_bass_kernel_spmd(nc, in_maps, list(range(N_CORES)), **kw)
    if trace:
        print(f"HW exec time: {res.exec_time_ns} ns")
        _COMPILED["last_result"] = res
    outs = [np.asarray(res.results[c]["out"], dtype=np.float32)
            for c in range(N_CORES)]
    return np.concatenate(outs, axis=0).reshape(B, L, DIMS)



# revision 3
# speedup vs baseline: 1.4172x; 1.4172x over previous
"""EfficientAttention Trainium2 Bass kernel.

Reference computation (per token t, H=16 heads, hd=64):
  Q = x @ Wq.T ; K = x @ Wk.T ; V = x @ Wv.T        (d = 1024)
  sK = softmax over heads of K^T      : sK[d,h] = expK[h,d] / rk[d]
  tran_V = sK @ V                      (64 x 64)
  out = softmax(Q, axis=-1) @ tran_V   (16 x 64) -> flatten 1024

Equivalent form used here (per token):
  A^T[h',h] = sum_d expQ[h,d] * sK[d,h']    (then scale rows by 1/rq[h])
  out[h,:]  = sum_h' A[h,h'] * V[h',:]

Sharding: data-parallel over the 16384 tokens across 8 cores (2048 each).
Weights replicated, pre-transposed + bf16-cast on host.

Device layout: tokens on SBUF partitions (128-token tiles).
  PE   : x-tile transposes + the three 1024x1024 projections (bf16)
  ACT  : psum->sbuf evictions fused with exp for Q,K
  DVE  : softmax normalizers + the per-token small matmuls
"""

import numpy as np
import ml_dtypes
from contextlib import ExitStack

import concourse.bass as bass
import concourse.mybir as mybir
import concourse.tile as tile
from concourse import bacc
from concourse.bass_utils import run_bass_kernel_spmd

DIMS = 1024
HEADS = 16
HD = 64
N_CORES = 8
B, L = 4, 4096
TOKENS = B * L
TOK_PER_CORE = TOKENS // N_CORES  # 2048
P = 128                           # tokens per tile (SBUF partitions)
N_TILES = TOK_PER_CORE // P       # 16

FP32 = mybir.dt.float32
BF16 = mybir.dt.bfloat16

_COMPILED = {}


def _register_mac_scan():
    """Custom fused DVE op: out = prefix-sum of (in0 * in1) along the stream.

    Used for the per-token second matmul: per head h, stream (e outer,
    h' inner) of V[t,(h',e)] * A[t,(h,h')]; segment sums are recovered by
    differencing the prefix at 16-element boundaries.
    """
    from concourse.dve_spec import Spec, Src0, Src1, scan, AluOp, lower
    from concourse import dve_ops
    from concourse.dve_uop import DveOpSpec
    from concourse.bass import dve_ver_for

    if "MAC_SCAN" in dve_ops._SUB_OPCODE_FOR_NAME:
        return next(o for o in dve_ops.OPS if o.name == "MAC_SCAN")

    spec = Spec(body=scan(AluOp.ADD, Src0 * Src1),
                reference=lambda in0, in1: np.cumsum(
                    in0.astype(np.float32) * in1.astype(np.float32), axis=-1))
    row = 1 + len(dve_ops.OPS)
    shas = {}
    for ver in ("v3", "v4"):
        tmp = DveOpSpec(name="MAC_SCAN", opcode=row,
                        uops=lower(spec, ver=ver), rd1_en=True)
        shas[ver] = tmp.sha(ver)
    op = dve_ops.DveOp("MAC_SCAN", spec, subdim=False, uops_sha=shas)
    dve_ops.OPS.append(op)
    dve_ops.CUSTOM_DVE_SPECS["MAC_SCAN"] = spec
    dve_ops._SUB_OPCODE_FOR_NAME["MAC_SCAN"] = row
    return op


MAC_SCAN = _register_mac_scan()


def _build_kernel():
    nc = bacc.Bacc("TRN2", target_bir_lowering=False)

    x_in = nc.dram_tensor("x", [TOK_PER_CORE, DIMS], BF16, kind="ExternalInput")
    wq_in = nc.dram_tensor("wq", [DIMS, DIMS], BF16, kind="ExternalInput")
    wk_in = nc.dram_tensor("wk", [DIMS, DIMS], BF16, kind="ExternalInput")
    wv_in = nc.dram_tensor("wv", [DIMS, DIMS], BF16, kind="ExternalInput")
    ident_in = nc.dram_tensor("ident", [P, P], BF16, kind="ExternalInput")
    out_d = nc.dram_tensor("out", [TOK_PER_CORE, DIMS], FP32, kind="ExternalOutput")

    with tile.TileContext(nc) as tc, ExitStack() as ctx:
        consts = ctx.enter_context(tc.tile_pool(name="consts", bufs=1))
        wpool = ctx.enter_context(tc.tile_pool(name="weights", bufs=1))
        xpool = ctx.enter_context(tc.tile_pool(name="x", bufs=N_TILES))
        xtpool = ctx.enter_context(tc.tile_pool(name="xt", bufs=2))
        smpool = ctx.enter_context(tc.tile_pool(name="sm", bufs=3))
        slabpool = ctx.enter_context(tc.tile_pool(name="slab", bufs=2))
        mmpool = ctx.enter_context(tc.tile_pool(name="mm", bufs=4))
        opool = ctx.enter_context(tc.tile_pool(name="outs", bufs=3))
        # PSUM budget (8 banks): pp(proj)=2, tp(xT+extract slabs)=2x2,
        # pa(A + A-back)=2
        ps_pp = ctx.enter_context(tc.tile_pool(name="ps_pp", bufs=2, space="PSUM"))
        ps_tp = ctx.enter_context(tc.tile_pool(name="ps_tp", bufs=4, space="PSUM"))
        ps_pa = ctx.enter_context(tc.tile_pool(name="ps_pa", bufs=2, space="PSUM"))

        ident = consts.tile([P, P], BF16)
        nc.sync.dma_start(ident[:], ident_in[:])

        xts = {}
        for pre in range(2):
            xt0 = xpool.tile([P, DIMS], BF16, tag="xt", name=f"xpre{pre}")
            nc.sync.dma_start(xt0[:], x_in[pre * P:(pre + 1) * P, :])
            xts[pre] = xt0

        ws = {}
        for name, w_in in (("q", wq_in), ("k", wk_in), ("v", wv_in)):
            w = wpool.tile([P, 8 * DIMS], BF16, tag=f"w{name}")
            for c in range(8):
                nc.sync.dma_start(w[:, c * DIMS:(c + 1) * DIMS],
                                  w_in[c * P:(c + 1) * P, :])
            ws[name] = w

        for it in range(N_TILES):
            # 1) load x tile [128 tokens, 1024] bf16
            if it in xts:
                xt = xts[it]
            else:
                xt = xpool.tile([P, DIMS], BF16)
                nc.sync.dma_start(xt[:], x_in[it * P:(it + 1) * P, :])

            # 2) transpose x tile -> xT [j, t] stored as 8 chunks side by side
            xT = xtpool.tile([P, DIMS], BF16)
            for c in range(8):
                tp = ps_tp.tile([P, P], BF16, tag="tp", name=f"tp{it}_{c}")
                nc.tensor.transpose(tp[:], xt[:, c * P:(c + 1) * P], ident[:])
                nc.scalar.copy(xT[:, c * P:(c + 1) * P], tp[:])

            # 3) projections; per proj 2 psum banks, evicted via ACT
            #    (exp fused for Q,K; plain copy for V)
            expq = smpool.tile([P, DIMS], BF16, tag="expq")
            expk = smpool.tile([P, DIMS], BF16, tag="expk")
            vt = smpool.tile([P, DIMS], BF16, tag="vt")
            for pname, dst, func in (
                ("q", expq, mybir.ActivationFunctionType.Exp),
                ("k", expk, mybir.ActivationFunctionType.Exp),
                ("v", vt, None),
            ):
                w = ws[pname]
                for nb in range(2):
                    pp = ps_pp.tile([P, 512], FP32, tag="pp",
                                    name=f"pp{it}_{pname}{nb}")
                    for c in range(8):
                        nc.tensor.matmul(
                            pp[:],
                            lhsT=xT[:, c * P:(c + 1) * P],
                            rhs=w[:, c * DIMS + nb * 512: c * DIMS + nb * 512 + 512],
                            start=(c == 0), stop=(c == 7),
                        )
                    sl = slice(nb * 512, nb * 512 + 512)
                    if func is None:
                        nc.scalar.copy(dst[:, sl], pp[:])
                    else:
                        nc.scalar.activation(dst[:, sl], pp[:], func)

            # 4) softmax normalizers on DVE
            rq = smpool.tile([P, HEADS], FP32, tag="rq")       # sum_d expQ[h,d]
            nc.vector.reduce_sum(rq[:], expq[:].rearrange("p (h d) -> p h d", d=HD),
                                 axis=mybir.AxisListType.X)
            # rk[d] = sum_h expK[t,(h,d)] via contiguous halving adds
            t1 = smpool.tile([P, 512], BF16, tag="t1")
            nc.vector.tensor_add(t1[:], expk[:, 0:512], expk[:, 512:1024])
            t2 = smpool.tile([P, 256], BF16, tag="t2")
            nc.vector.tensor_add(t2[:], t1[:, 0:256], t1[:, 256:512])
            t3 = smpool.tile([P, 128], BF16, tag="t3")
            nc.vector.tensor_add(t3[:], t2[:, 0:128], t2[:, 128:256])
            rk = smpool.tile([P, HD], FP32, tag="rk")
            nc.vector.tensor_add(rk[:], t3[:, 0:HD], t3[:, HD:128])
            rqi = smpool.tile([P, HEADS], FP32, tag="rqi")
            nc.vector.reciprocal_approx_fast(rqi[:], rq[:])
            rki = smpool.tile([P, HD], FP32, tag="rki")
            nc.vector.reciprocal_approx_fast(rki[:], rk[:])
            rkib = smpool.tile([P, HD], BF16, tag="rkib")
            nc.scalar.copy(rkib[:], rki[:])

            # 5) sQ'[t,(h,d)] = expQ * rqi[h] * rki[d]  (both softmax scales
            #    folded into the Q side; K side stays raw expK)
            sqt = smpool.tile([P, DIMS], BF16, tag="sqt")
            rkib_b = rkib[:].unsqueeze(1).broadcast_to([P, HEADS, HD])
            nc.vector.tensor_mul(sqt[:].rearrange("p (h d) -> p h d", d=HD),
                                 expq[:].rearrange("p (h d) -> p h d", d=HD),
                                 rkib_b)

            # 6) extraction: per-head PE transposes -> feature-on-partition
            #    slabs QS/KS [64 d, 16 heads x 128 tokens] bf16
            slabs = {}
            for sname, srct in (("qs", sqt), ("ks", expk)):
                slab = slabpool.tile([HD, HEADS * P], BF16, tag=sname)
                for b in range(2):
                    ep = ps_tp.tile([HD, 8 * P], BF16, tag="tp",
                                    name=f"ep{it}_{sname}{b}")
                    for hh in range(8):
                        h = 8 * b + hh
                        nc.tensor.transpose(
                            ep[:, hh * P:(hh + 1) * P],
                            srct[:, h * HD:(h + 1) * HD],
                            ident[:])
                    nc.scalar.copy(slab[:, b * 8 * P:(b + 1) * 8 * P], ep[:])
                slabs[sname] = slab

            # 7) mm1 on PE: per token A[h',h] = sum_d KS[d,h'] * QS[d,h]
            #    out [16,16] per token, 32 tokens per psum bank, evicted bf16
            aev = mmpool.tile([HEADS, 2048], BF16, tag="aev")
            for bk in range(4):
                pa = ps_pa.tile([HEADS, 512], FP32, tag="pa",
                                name=f"pa{it}_{bk}")
                for ts in range(32):
                    t = 32 * bk + ts
                    lhsT = slabs["ks"][:, t::P]
                    rhs = slabs["qs"][:, t::P]
                    nc.tensor.matmul(
                        pa[:, ts * HEADS:(ts + 1) * HEADS],
                        lhsT=lhsT, rhs=rhs,
                        start=True, stop=True)
                nc.scalar.copy(aev[:, bk * 512:(bk + 1) * 512], pa[:])

            # 8) A back to token-partitions: per h transpose
            #    [16 h', 128 t] -> [128 t, 16 h'], pack into at [128, 256]
            at = mmpool.tile([P, HEADS * HEADS], BF16, tag="at")
            ab = ps_pa.tile([P, HEADS * HEADS], BF16, tag="pa",
                            name=f"ab{it}")
            for h in range(HEADS):
                src = aev[:, h::HEADS]
                nc.tensor.transpose(
                    ab[:, h * HEADS:(h + 1) * HEADS],
                    src, ident[0:HEADS, 0:HEADS])
            nc.scalar.copy(at[:], ab[:])
            rqi_b = rqi[:].unsqueeze(2).broadcast_to([P, HEADS, HEADS])
            nc.vector.tensor_mul(at[:].rearrange("p (h k) -> p h k", k=HEADS),
                                 at[:].rearrange("p (h k) -> p h k", k=HEADS),
                                 rqi_b)

            # 9) mm2 via MAC-scan: out[t,(h,e)] = sum_h' A[t,(h,h')]*V[t,(h',e)]
            #    vt is host-permuted to (e outer, h' inner) so in0 streams
            #    contiguously.
            ot = opool.tile([P, DIMS], FP32, tag="ot")
            sc = mmpool.tile([P, HEADS * HD + 1], FP32, tag="sc")
            nc.vector.memset(sc[:, 0:1], 0.0)
            for h in range(HEADS):
                a_view = at[:, h * HEADS:(h + 1) * HEADS] \
                    .unsqueeze(1).broadcast_to([P, HD, HEADS])
                nc.vector._custom_dve(
                    MAC_SCAN,
                    out=sc[:, 1:HEADS * HD + 1],
                    in0=vt[:],
                    in1=a_view,
                )
                nc.vector.tensor_sub(
                    ot[:, h * HD:(h + 1) * HD],
                    sc[:, HEADS:HEADS * HD + 1:HEADS],
                    sc[:, 0:HEADS * HD:HEADS],
                )
            # 10) store
            nc.sync.dma_start(out_d[it * P:(it + 1) * P, :], ot[:])

    nc.compile()
    return nc


def kernel(input_seq_embs, W_Q, W_K, W_V):
    x = np.asarray(input_seq_embs, dtype=np.float32).reshape(TOKENS, DIMS)
    x_bf = x.astype(ml_dtypes.bfloat16)
    # torch Linear computes x @ W.T; our matmul wants rhs = W.T laid out
    # [contraction j, out i] == W_Q.T, which is exactly W.T in row-major.
    wq = np.ascontiguousarray(np.asarray(W_Q, np.float32).T).astype(ml_dtypes.bfloat16)
    wk = np.ascontiguousarray(np.asarray(W_K, np.float32).T).astype(ml_dtypes.bfloat16)
    # W_V's output features are permuted (h', e) -> (e, h') so the on-device
    # V tile streams (e outer, h' inner) contiguously for the MAC-scan.
    wv_f = np.asarray(W_V, np.float32).reshape(HEADS, HD, DIMS)
    wv = np.ascontiguousarray(
        wv_f.transpose(2, 1, 0).reshape(DIMS, DIMS)).astype(ml_dtypes.bfloat16)
    ident = np.eye(P, dtype=ml_dtypes.bfloat16)

    if "nc" not in _COMPILED:
        _COMPILED["nc"] = _build_kernel()
    nc = _COMPILED["nc"]

    in_maps = []
    for c in range(N_CORES):
        shard = np.ascontiguousarray(x_bf[c * TOK_PER_CORE:(c + 1) * TOK_PER_CORE])
        in_maps.append({"x": shard, "wq": wq, "wk": wk, "wv": wv, "ident": ident})

    import os
    trace = bool(int(os.environ.get("KERNEL_PROFILE", "0")))
    kw = {}
    if trace:
        kw = dict(trace=True, tmpdir=os.environ.get("KERNEL_TRACE_DIR") or None)
    res = run_bass_kernel_spmd(nc, in_maps, list(range(N_CORES)), **kw)
    if trace:
        print(f"HW exec time: {res.exec_time_ns} ns")
        _COMPILED["last_result"] = res
    outs = [np.asarray(res.results[c]["out"], dtype=np.float32)
            for c in range(N_CORES)]
    return np.concatenate(outs, axis=0).reshape(B, L, DIMS)

